# revision 2
# baseline (speedup 1.0000x reference)
"""GateRecurrent2dnoind (horizontal, forward) Trainium2 kernel.

Semantics (matching the reference):
  G1u, G2u = bilinear 2x upsample (half-pixel) of G1, G2 to (256, 256)
  g1x = G1u * X
  o = g1x; repeat 128x: o = g1x + G2u * shift_right_w(o)   (left edge replicated)

The 128 Jacobi passes are computed as ONE sequential scan along W:
  s[x] = g1x[x] + G2u[x] * s[x-1]
with an exact depth-128 window emulation:
  - boundary init: s[-1] = b0 * sum_{m=0}^{127} a0^m   (a0 = G2u[...,0], b0 = g1x[...,0])
  - window correction: data1[x] -= q[x] for x in 1..K, where
      q[x] = b0 * a0^(129-x) * prod_{i=1..x} G2u[...,i]
    computed by a second scan q[x] = (G2u[x]/a0) * q[x-1], q-init = b0*a0^129,
    floored to 0 when a0 < 0.5 (correction < 0.5^65 there, far below f32).
    The main scan then propagates q into exactly the missing window terms.

Sharding: batch b -> core b (8 batches, 8 cores). Per core: [64, 256, 256].
Layout: H on partitions (even/odd output-row parities as separate tiles, so the
2x H-upsample is two 128x128 matmuls), W and a 4-channel block on the free dim.
"""

import contextlib

import numpy as np

import concourse.bacc as bacc
import concourse.bass as bass
import concourse.mybir as mybir
import concourse.tile as tile
from concourse.bass_utils import run_bass_kernel_spmd

f32 = mybir.dt.float32
Alu = mybir.AluOpType

NCORES = 8
C = 64          # channels per core
H = 256
W = 256
HG = 128        # G input h/w
B = 4           # channels per block
NBLK = C // B
K = 64          # correction columns


def _upsample_mats():
    """lhsT [k=in_row, m=out_row] for the H-upsample matmuls, scaled by 0.25.

    even rows: out[m] = 0.25*in[m-1] + 0.75*in[m]   (m=0 clamps to in[0])
    odd rows:  out[m] = 0.75*in[m] + 0.25*in[m+1]   (m=127 clamps to in[127])
    """
    ue = np.zeros((HG, HG), np.float32)
    uo = np.zeros((HG, HG), np.float32)
    for m in range(HG):
        ue[m, m] += 0.25 * 0.75
        ue[max(m - 1, 0), m] += 0.25 * 0.25
        uo[m, m] += 0.25 * 0.75
        uo[min(m + 1, HG - 1), m] += 0.25 * 0.25
    return ue, uo


def _emit(nc, pools, ue, uo, dram):
    pcp, pcps, ginp, psp, hupp, gupp, xinp, datp, qtp, outp = pools
    Xd, G1d, G2d, Od = dram

    # ---- per-core boundary precompute (batched over all channels) ------
    g2c0 = pcp.tile([HG, C], f32, tag="g2c0")
    nc.sync.dma_start(g2c0[:], G2d[:, :, 0].transpose([1, 0]))
    coefs = {}
    for par, u in (("e", ue), ("o", uo)):
        ps = pcps.tile([HG, C], f32, tag="pcps")
        nc.tensor.matmul(ps[:], u[:], g2c0[:], start=True, stop=True)
        a0 = pcp.tile([HG, C], f32, tag=f"a0{par}")
        nc.vector.tensor_scalar_mul(a0[:], ps[:], 4.0)
        # geo = sum_{m=0}^{127} a0^m = prod_k (1 + a0^(2^k)), k=0..6
        acc = pcp.tile([HG, C], f32, tag=f"acc{par}")
        p = pcp.tile([HG, C], f32, tag=f"p{par}")
        t = pcp.tile([HG, C], f32, tag=f"t{par}")
        nc.vector.tensor_scalar_add(acc[:], a0[:], 1.0)
        nc.vector.tensor_tensor(p[:], a0[:], a0[:], Alu.mult)
        for _ in range(5):
            nc.vector.tensor_scalar_add(t[:], p[:], 1.0)
            nc.vector.tensor_tensor(acc[:], acc[:], t[:], Alu.mult)
            nc.vector.tensor_tensor(p[:], p[:], p[:], Alu.mult)
        nc.vector.tensor_scalar_add(t[:], p[:], 1.0)
        nc.vector.tensor_tensor(acc[:], acc[:], t[:], Alu.mult)
        a128 = pcp.tile([HG, C], f32, tag=f"a128{par}")
        nc.vector.tensor_tensor(a128[:], p[:], p[:], Alu.mult)
        # s0coef = 1 + a0*geo  (s[0] = b0*s0coef)
        s0c = pcp.tile([HG, C], f32, tag=f"s0c{par}")
        nc.vector.tensor_tensor(t[:], a0[:], acc[:], Alu.mult)
        nc.vector.tensor_scalar_add(s0c[:], t[:], 1.0)
        # qcoef = mask(a0>=0.5) * a128 * a0   (q-init = b0*qcoef)
        mask = pcp.tile([HG, C], f32, tag=f"mask{par}")
        nc.vector.tensor_scalar(mask[:], a0[:], 0.5, None, Alu.is_ge)
        rec = pcp.tile([HG, C], f32, tag=f"rec{par}")
        nc.vector.tensor_scalar_max(t[:], a0[:], 0.5)
        nc.vector.reciprocal(rec[:], t[:])
        qc = pcp.tile([HG, C], f32, tag=f"qc{par}")
        nc.vector.tensor_tensor(qc[:], mask[:], a128[:], Alu.mult)
        nc.vector.tensor_tensor(qc[:], qc[:], a0[:], Alu.mult)
        coefs[par] = (s0c, qc, rec)

    # ---- main loop -----------------------------------------------------
    for blk in range(NBLK):
        c0 = blk * B
        g1b = ginp.tile([HG, B * HG], f32, tag="g1b")
        g2b = ginp.tile([HG, B * HG], f32, tag="g2b")
        nc.sync.dma_start(
            g1b[:].rearrange("p (c w) -> p c w", c=B),
            G1d[c0:c0 + B, :, :].transpose([1, 0, 2]))
        nc.sync.dma_start(
            g2b[:].rearrange("p (c w) -> p c w", c=B),
            G2d[c0:c0 + B, :, :].transpose([1, 0, 2]))
        for par, u in (("e", ue), ("o", uo)):
            s0c, qc, rec = coefs[par]
            pstart = 0 if par == "e" else 1
            # H-upsample (PE): A = 0.25 * Hup  [128, (B,128)]
            a1 = psp.tile([HG, B * HG], f32, tag="a1")
            a2 = psp.tile([HG, B * HG], f32, tag="a2")
            nc.tensor.matmul(a1[:], u[:], g1b[:], start=True, stop=True)
            nc.tensor.matmul(a2[:], u[:], g2b[:], start=True, stop=True)
            c1 = hupp.tile([HG, B * HG], f32, tag="c1")
            c2 = hupp.tile([HG, B * HG], f32, tag="c2")
            c2x3 = hupp.tile([HG, B * HG], f32, tag="c2x3")
            nc.scalar.copy(c1[:], a1[:])
            nc.scalar.copy(c2[:], a2[:])
            nc.scalar.mul(c2x3[:], a2[:], 3.0)

            # W-upsample: out[2j] = 3*q[j] + q[j-1]; out[2j+1] = 3*q[j] + q[j+1]
            g1u = gupp.tile([HG, B * W], f32, tag="g1u")
            g2u = gupp.tile([HG, B * W], f32, tag="g2u")
            c1r = c1[:].rearrange("p (c w) -> p c w", c=B)
            c2r = c2[:].rearrange("p (c w) -> p c w", c=B)
            c23r = c2x3[:].rearrange("p (c w) -> p c w", c=B)
            g1r = g1u[:].rearrange("p (c w) -> p c w", c=B)
            g2r = g2u[:].rearrange("p (c w) -> p c w", c=B)
            # G1 on vector (scalar_tensor_tensor: (q*3) + q_shift)
            nc.vector.scalar_tensor_tensor(
                g1r[:, :, 2:W:2], c1r[:, :, 1:HG], 3.0,
                c1r[:, :, 0:HG - 1], Alu.mult, Alu.add)
            nc.vector.scalar_tensor_tensor(
                g1r[:, :, 1:W - 1:2], c1r[:, :, 0:HG - 1], 3.0,
                c1r[:, :, 1:HG], Alu.mult, Alu.add)
            nc.vector.scalar_tensor_tensor(
                g1r[:, :, 0:1], c1r[:, :, 0:1], 3.0,
                c1r[:, :, 0:1], Alu.mult, Alu.add)
            nc.vector.scalar_tensor_tensor(
                g1r[:, :, W - 1:W], c1r[:, :, HG - 1:HG], 3.0,
                c1r[:, :, HG - 1:HG], Alu.mult, Alu.add)
            # G2 on gpsimd (tensor_tensor only: out = 3q + q_shift);
            # col 0 must be 0 (scan re-init at channel seam)
            nc.gpsimd.tensor_tensor(
                g2r[:, :, 2:W:2], c23r[:, :, 1:HG],
                c2r[:, :, 0:HG - 1], Alu.add)
            nc.gpsimd.tensor_tensor(
                g2r[:, :, 1:W - 1:2], c23r[:, :, 0:HG - 1],
                c2r[:, :, 1:HG], Alu.add)
            nc.gpsimd.memset(g2r[:, :, 0:1], 0.0)
            nc.gpsimd.tensor_tensor(
                g2r[:, :, W - 1:W], c23r[:, :, HG - 1:HG],
                c2r[:, :, HG - 1:HG], Alu.add)

            # X block for this parity
            xb = xinp.tile([HG, B * W], f32, tag="xb")
            nc.sync.dma_start(
                xb[:].rearrange("p (c w) -> p c w", c=B),
                Xd[c0:c0 + B, pstart:H:2, :].transpose([1, 0, 2]))

            # data1 = g1x
            d = datp.tile([HG, B * W], f32, tag="d")
            nc.vector.tensor_tensor(d[:], g1u[:], xb[:], Alu.mult)
            dr = d[:].rearrange("p (c w) -> p c w", c=B)

            # window correction scan over cols 1..K
            recb = qtp.tile([HG, B * K], f32, tag="recb")
            recb_r = recb[:].rearrange("p (c w) -> p c w", c=B)
            nc.vector.tensor_copy(
                recb_r[:, :, :],
                rec[:, c0:c0 + B].unsqueeze(-1).to_broadcast([HG, B, K]))
            qd = qtp.tile([HG, B * (K + 1)], f32, tag="qd")
            qz = qtp.tile([HG, B * (K + 1)], f32, tag="qz")
            qo = qtp.tile([HG, B * (K + 1)], f32, tag="qo")
            qdr = qd[:].rearrange("p (c w) -> p c w", c=B)
            qzr = qz[:].rearrange("p (c w) -> p c w", c=B)
            qor = qo[:].rearrange("p (c w) -> p c w", c=B)
            nc.gpsimd.memset(qdr[:, :, 0:1], 0.0)
            nc.gpsimd.tensor_tensor(
                qdr[:, :, 1:K + 1], g2r[:, :, 1:K + 1], recb_r[:, :, :],
                Alu.mult)
            nc.gpsimd.memset(qz[:], 0.0)
            # spacer data1 = b0*qcoef; b0 = d[:, (c, 0)] (pre-overwrite)
            nc.vector.tensor_tensor(
                qzr[:, :, 0:1], dr[:, :, 0:1],
                qc[:, c0:c0 + B].unsqueeze(-1), Alu.mult)
            # d col0 = b0 * s0coef (in place, after qz spacer read)
            nc.vector.tensor_tensor(
                dr[:, :, 0:1], dr[:, :, 0:1],
                s0c[:, c0:c0 + B].unsqueeze(-1), Alu.mult)
            nc.vector.tensor_tensor_scan(
                qo[:], qd[:], qz[:], 0.0, Alu.mult, Alu.add)
            nc.gpsimd.tensor_tensor(
                dr[:, :, 1:K + 1], dr[:, :, 1:K + 1], qor[:, :, 1:K + 1],
                Alu.subtract)

            # main scan
            ot = outp.tile([HG, B * W], f32, tag="ot")
            nc.vector.tensor_tensor_scan(
                ot[:], g2u[:], d[:], 0.0, Alu.mult, Alu.add)
            nc.sync.dma_start(
                Od[c0:c0 + B, pstart:H:2, :].transpose([1, 0, 2]),
                ot[:].rearrange("p (c w) -> p c w", c=B))


def build(loop_n=None):
    nc = bacc.Bacc("TRN2", target_bir_lowering=False, debug=False,
                   num_devices=NCORES)
    Xd = nc.dram_tensor("X", [C, H, W], f32, kind="ExternalInput")
    G1d = nc.dram_tensor("G1", [C, HG, HG], f32, kind="ExternalInput")
    G2d = nc.dram_tensor("G2", [C, HG, HG], f32, kind="ExternalInput")
    UEd = nc.dram_tensor("UE", [HG, HG], f32, kind="ExternalInput")
    UOd = nc.dram_tensor("UO", [HG, HG], f32, kind="ExternalInput")
    Od = nc.dram_tensor("O", [C, H, W], f32, kind="ExternalOutput")

    with tile.TileContext(nc) as tc:
        with (
            tc.tile_pool(name="const", bufs=1) as constp,
            tc.tile_pool(name="pc", bufs=1) as pcp,
            tc.tile_pool(name="pcps", bufs=2, space="PSUM") as pcps,
            tc.tile_pool(name="gin", bufs=3) as ginp,
            tc.tile_pool(name="ps", bufs=2, space="PSUM") as psp,
            tc.tile_pool(name="hupc", bufs=3) as hupp,
            tc.tile_pool(name="gup", bufs=3) as gupp,
            tc.tile_pool(name="xin", bufs=3) as xinp,
            tc.tile_pool(name="dat", bufs=3) as datp,
            tc.tile_pool(name="qt", bufs=3) as qtp,
            tc.tile_pool(name="out", bufs=3) as outp,
        ):
            ue = constp.tile([HG, HG], f32, tag="ue")
            uo = constp.tile([HG, HG], f32, tag="uo")
            nc.sync.dma_start(ue[:], UEd[:])
            nc.sync.dma_start(uo[:], UOd[:])

            pools = (pcp, pcps, ginp, psp, hupp, gupp, xinp, datp, qtp, outp)
            dram = (Xd, G1d, G2d, Od)
            if loop_n:
                with tc.For_i(0, loop_n, 1):
                    _emit(nc, pools, ue, uo, dram)
            else:
                _emit(nc, pools, ue, uo, dram)

    nc.compile()
    return nc


_NC = None


def kernel(X, G1, G2, G3=None, **_):
    global _NC
    if _NC is None:
        _NC = build()
    ue, uo = _upsample_mats()
    in_maps = [
        {"X": np.ascontiguousarray(X[k]), "G1": np.ascontiguousarray(G1[k]),
         "G2": np.ascontiguousarray(G2[k]), "UE": ue, "UO": uo}
        for k in range(NCORES)
    ]
    import os
    res = run_bass_kernel_spmd(_NC, in_maps, list(range(NCORES)),
                               tmpdir=os.environ.get("KERNEL_TMPDIR"))
    kernel.last_result = res
    out = np.stack([res.results[k]["O"] for k in range(NCORES)])
    return out.astype(np.float32, copy=False)



# revision 9
# speedup vs baseline: 1.4772x; 1.4772x over previous
"""GateRecurrent2dnoind (horizontal, forward) Trainium2 kernel, v2.

Semantics (matching the reference):
  G1u, G2u = bilinear 2x upsample (half-pixel) of G1, G2 to (256, 256)
  g1x = G1u * X
  o = g1x; repeat 128x: o = g1x + G2u * shift_right_w(o)   (left edge replicated)

The 128 Jacobi passes equal ONE sequential scan along W:
  s[x] = d[x] + a[x] * s[x-1]      a = G2u, d = G1u*X
with exact depth-128 window emulation (boundary init via geometric series +
a K-column correction scan), identical math to the validated v1 kernel.

v2 engine plan (from measured rates: DVE TT 1.12 ns/el, scan 2.1 ns/el any
dtype/stride, scalar ACTIVATE 1.05 ns/el incl strided+PSUM, gpsimd TT 4.6,
fp32 matmul 592+360(LDW) ns per 512 cols):
  - half-res W-combine m_e[j]=3g[j]+g[j-1], m_o[j]=3g[j]+g[j+1] on DVE (G2)
    and GPSIMD (G1, using a scalar-built 3*g1), from halo'd g tiles.
  - H-upsample = ONE fp32 matmul per (parity, w-parity, 4ch chunk) on PE.
  - scalar engine drains PSUM -> interleaved full-res a~/b~ (strided f32).
  - DVE: d~ = b~*X, correction scan (K cols), then one full-length scan per
    parity writing the output tile directly.

Sharding: batch b -> core b (8 batches, 8 cores). Per core: [64, 256, 256].
"""

import numpy as np

import concourse.bacc as bacc
import concourse.bass as bass
import concourse.mybir as mybir
import concourse.tile as tile
from concourse.bass_utils import run_bass_kernel_spmd

f32 = mybir.dt.float32
Alu = mybir.AluOpType

NCORES = 8
C = 64          # channels per core
H = 256
W = 256
HG = 128        # G input h/w
BC = 8          # channels per block
NBLK = C // BC  # 8
K = 32          # correction columns
SLOT = HG + 2   # g tile slot width (halo col on each side)


def _upsample_mats():
    """lhsT [k=in_row, m=out_row] for the H-upsample matmuls, scaled by 0.25.

    even rows: out[m] = 0.25*in[m-1] + 0.75*in[m]   (m=0 clamps to in[0])
    odd rows:  out[m] = 0.75*in[m] + 0.25*in[m+1]   (m=127 clamps to in[127])
    """
    ue = np.zeros((HG, HG), np.float32)
    uo = np.zeros((HG, HG), np.float32)
    for m in range(HG):
        ue[m, m] += 0.25 * 0.75
        ue[max(m - 1, 0), m] += 0.25 * 0.25
        uo[m, m] += 0.25 * 0.75
        uo[min(m + 1, HG - 1), m] += 0.25 * 0.25
    return ue, uo


def _precompute(nc, pcp, psp, us, G2d):
    """Boundary coefficients per parity from a0 = G2u[..., 0].

    Returns combined [128, 2*C] tiles s0cB, qcB and recbb [128, 2*C*K]:
      s0c = 1 + a0*sum_{m=0}^{127} a0^m          (s[0] = b0*s0c)
      qc  = mask(a0>=0.5) * a0^129               (q-init = b0*qc)
      rec = 1/max(a0, 0.5)  broadcast over K cols into recbb
    """
    g2c0 = pcp.tile([HG, C], f32, tag="g2c0")
    nc.sync.dma_start(g2c0[:], G2d[:, :, 0].transpose([1, 0]))
    s0cB = pcp.tile([HG, 2 * C], f32, tag="s0cB")
    qcB = pcp.tile([HG, 2 * C], f32, tag="qcB")
    recbb = pcp.tile([HG, 2 * C * K], f32, tag="recbb")
    for par in (0, 1):
        ps = psp.tile([HG, C], f32, tag="pc")
        nc.tensor.matmul(ps[:], us[par][:], g2c0[:], start=True, stop=True)
        a0 = pcp.tile([HG, C], f32, tag=f"a0{par}")
        nc.vector.tensor_scalar_mul(a0[:], ps[:], 4.0)
        # geo = sum_{m=0}^{127} a0^m = prod_k (1 + a0^(2^k)), k=0..6
        acc = pcp.tile([HG, C], f32, tag=f"acc{par}")
        p = pcp.tile([HG, C], f32, tag=f"p{par}")
        t = pcp.tile([HG, C], f32, tag=f"t{par}")
        nc.vector.tensor_scalar_add(acc[:], a0[:], 1.0)
        nc.vector.tensor_tensor(p[:], a0[:], a0[:], Alu.mult)
        for _ in range(5):
            nc.vector.tensor_scalar_add(t[:], p[:], 1.0)
            nc.vector.tensor_tensor(acc[:], acc[:], t[:], Alu.mult)
            nc.vector.tensor_tensor(p[:], p[:], p[:], Alu.mult)
        nc.vector.tensor_scalar_add(t[:], p[:], 1.0)
        nc.vector.tensor_tensor(acc[:], acc[:], t[:], Alu.mult)
        a128 = pcp.tile([HG, C], f32, tag=f"a128{par}")
        nc.vector.tensor_tensor(a128[:], p[:], p[:], Alu.mult)
        # s0c = 1 + a0*geo
        nc.vector.tensor_tensor(t[:], a0[:], acc[:], Alu.mult)
        nc.vector.tensor_scalar_add(s0cB[:, par * C:(par + 1) * C], t[:], 1.0)
        # qc = mask(a0>=0.5) * a128 * a0
        mask = pcp.tile([HG, C], f32, tag=f"mask{par}")
        nc.vector.tensor_scalar(mask[:], a0[:], 0.5, None, Alu.is_ge)
        rec = pcp.tile([HG, C], f32, tag=f"rec{par}")
        nc.vector.tensor_scalar_max(t[:], a0[:], 0.5)
        nc.vector.reciprocal(rec[:], t[:])
        qc = pcp.tile([HG, C], f32, tag=f"qc{par}")
        nc.vector.tensor_tensor(qc[:], mask[:], a128[:], Alu.mult)
        nc.vector.tensor_tensor(qcB[:, par * C:(par + 1) * C], qc[:], a0[:],
                                Alu.mult)
        nc.vector.tensor_copy(
            recbb[:].rearrange("p (q c k) -> p q c k", q=2, c=C)[:, par],
            rec[:].unsqueeze(-1).to_broadcast([HG, C, K]))
    return s0cB, qcB, recbb


def build():
    nc = bacc.Bacc("TRN2", target_bir_lowering=False, debug=False,
                   num_devices=NCORES)
    Xd = nc.dram_tensor("X", [C, H, W], f32, kind="ExternalInput")
    G1d = nc.dram_tensor("G1", [C, HG, HG], f32, kind="ExternalInput")
    G2d = nc.dram_tensor("G2", [C, HG, HG], f32, kind="ExternalInput")
    UEd = nc.dram_tensor("UE", [HG, HG], f32, kind="ExternalInput")
    UOd = nc.dram_tensor("UO", [HG, HG], f32, kind="ExternalInput")
    Od = nc.dram_tensor("O", [C, H, W], f32, kind="ExternalOutput")

    with tile.TileContext(nc) as tc:
        with (
            tc.tile_pool(name="const", bufs=1) as constp,
            tc.tile_pool(name="pc", bufs=1) as pcp,
            tc.tile_pool(name="psum", bufs=2, space="PSUM") as psp,
            tc.tile_pool(name="gin", bufs=2) as gpool,
            tc.tile_pool(name="m", bufs=2) as mpool,
            tc.tile_pool(name="ab", bufs=2) as abpool,
            tc.tile_pool(name="x", bufs=2) as xpool,
            tc.tile_pool(name="d", bufs=2) as dpool,
            tc.tile_pool(name="o", bufs=2) as opool,
            tc.tile_pool(name="q", bufs=2) as qpool,
        ):
            ue = constp.tile([HG, HG], f32, tag="ue")
            uo = constp.tile([HG, HG], f32, tag="uo")
            nc.sync.dma_start(ue[:], UEd[:])
            nc.sync.dma_start(uo[:], UOd[:])
            us = (ue, uo)

            s0cB, qcB, recbb = _precompute(nc, pcp, psp, us, G2d)
            s0cr = s0cB[:].rearrange("p (q c) -> p q c", q=2)
            qcr = qcB[:].rearrange("p (q c) -> p q c", q=2)
            recr = recbb[:].rearrange("p (q c k) -> p q c k", q=2, c=C)

            for b in range(NBLK):
                c0 = b * BC
                # ---- G loads with halo: slot s=0 -> g[0], 1..128 -> g,
                # s=129 -> g[127] ------------------------------------------
                gts = []
                for Gd, tag in ((G1d, "g1t"), (G2d, "g2t")):
                    gt = gpool.tile([HG, BC * SLOT], f32, tag=tag)
                    gv = gt[:].rearrange("p (c s) -> p c s", c=BC)
                    nc.sync.dma_start(
                        gv[:, :, 1:HG + 1],
                        Gd[c0:c0 + BC, :, :].transpose([1, 0, 2]))
                    nc.sync.dma_start(
                        gv[:, :, 0:1],
                        Gd[c0:c0 + BC, :, 0:1].transpose([1, 0, 2]))
                    nc.sync.dma_start(
                        gv[:, :, SLOT - 1:SLOT],
                        Gd[c0:c0 + BC, :, HG - 1:HG].transpose([1, 0, 2]))
                    gts.append((gt, gv))
                (g1t, g1v), (g2t, g2v) = gts

                # ---- half-res W-combines m_e=3g[j]+g[j-1], m_o=3g[j]+g[j+1]
                # G2 on DVE (STT); G1 on gpsimd (TT with scalar-built 3*g1).
                m2e = mpool.tile([HG, BC * HG], f32, tag="m2e")
                m2o = mpool.tile([HG, BC * HG], f32, tag="m2o")
                nc.vector.scalar_tensor_tensor(
                    m2e[:].rearrange("p (c j) -> p c j", c=BC),
                    g2v[:, :, 1:HG + 1], 3.0, g2v[:, :, 0:HG],
                    Alu.mult, Alu.add)
                nc.vector.scalar_tensor_tensor(
                    m2o[:].rearrange("p (c j) -> p c j", c=BC),
                    g2v[:, :, 1:HG + 1], 3.0, g2v[:, :, 2:HG + 2],
                    Alu.mult, Alu.add)
                g1x3 = mpool.tile([HG, BC * SLOT], f32, tag="g1x3")
                nc.scalar.mul(g1x3[:], g1t[:], 3.0)
                g1x3v = g1x3[:].rearrange("p (c s) -> p c s", c=BC)
                m1e = mpool.tile([HG, BC * HG], f32, tag="m1e")
                m1o = mpool.tile([HG, BC * HG], f32, tag="m1o")
                nc.gpsimd.tensor_tensor(
                    m1e[:].rearrange("p (c j) -> p c j", c=BC),
                    g1x3v[:, :, 1:HG + 1], g1v[:, :, 0:HG], Alu.add)
                nc.gpsimd.tensor_tensor(
                    m1o[:].rearrange("p (c j) -> p c j", c=BC),
                    g1x3v[:, :, 1:HG + 1], g1v[:, :, 2:HG + 2], Alu.add)

                dt = dpool.tile([HG, 2 * BC * W], f32, tag="dt")
                dtv = dt[:].rearrange("p (q c w) -> p q c w", q=2, c=BC)
                atiles = []
                for par in (0, 1):
                    # ---- PE H-upsample + scalar interleave drain ----------
                    at = abpool.tile([HG, BC * W], f32, tag="at")
                    bt = abpool.tile([HG, BC * W], f32, tag="bt")
                    for mt, dst, eo in ((m2e, at, 0), (m2o, at, 1),
                                        (m1e, bt, 0), (m1o, bt, 1)):
                        dv = dst[:].rearrange(
                            "p (c w2 two) -> p c w2 two", c=BC, two=2)
                        for chunk in range(2):
                            ps = psp.tile([HG, 4 * HG], f32,
                                          tag=f"ps{chunk}")
                            nc.tensor.matmul(
                                ps[:], us[par][:],
                                mt[:, chunk * 4 * HG:(chunk + 1) * 4 * HG],
                                start=True, stop=True)
                            nc.scalar.copy(
                                dv[:, chunk * 4:(chunk + 1) * 4, :, eo],
                                ps[:].rearrange("p (c j) -> p c j", c=4))
                    # seam restart: a[..., 0] = 0 per channel
                    atv = at[:].rearrange("p (c w) -> p c w", c=BC)
                    nc.gpsimd.memset(atv[:, :, 0:1], 0.0)

                    # ---- X load + d~ = b~ * X ----------------------------
                    xt = xpool.tile([HG, BC * W], f32, tag="xt")
                    nc.sync.dma_start(
                        xt[:].rearrange("p (c w) -> p c w", c=BC),
                        Xd[c0:c0 + BC, par:H:2, :].transpose([1, 0, 2]))
                    nc.vector.tensor_tensor(
                        dt[:, par * BC * W:(par + 1) * BC * W],
                        bt[:], xt[:], Alu.mult)
                    atiles.append((at, atv))

                # ---- corrections (chblock scope, both parities) ----------
                qd = qpool.tile([HG, 2 * BC * (K + 1)], f32, tag="qd")
                qz = qpool.tile([HG, 2 * BC * (K + 1)], f32, tag="qz")
                qo = qpool.tile([HG, 2 * BC * (K + 1)], f32, tag="qo")
                qdv = qd[:].rearrange("p (q c k) -> p q c k", q=2, c=BC)
                qzv = qz[:].rearrange("p (q c k) -> p q c k", q=2, c=BC)
                qov = qo[:].rearrange("p (q c k) -> p q c k", q=2, c=BC)
                nc.gpsimd.memset(qz[:], 0.0)
                nc.gpsimd.memset(qdv[:, :, :, 0:1], 0.0)
                for par in (0, 1):
                    nc.vector.tensor_tensor(
                        qdv[:, par, :, 1:K + 1],
                        atiles[par][1][:, :, 1:K + 1],
                        recr[:, par, c0:c0 + BC, :], Alu.mult)
                # qz spacer col0 = b0*qc (b0 = d~[...,0] pre-overwrite)
                nc.vector.tensor_tensor(
                    qzv[:, :, :, 0:1], dtv[:, :, :, 0:1],
                    qcr[:, :, c0:c0 + BC].unsqueeze(-1), Alu.mult)
                # d~ col0 = b0 * s0c (after qz spacer read)
                nc.vector.tensor_tensor(
                    dtv[:, :, :, 0:1], dtv[:, :, :, 0:1],
                    s0cr[:, :, c0:c0 + BC].unsqueeze(-1), Alu.mult)
                nc.vector.tensor_tensor_scan(
                    qo[:], qd[:], qz[:], 0.0, Alu.mult, Alu.add)
                nc.vector.tensor_tensor(
                    dtv[:, :, :, 1:K + 1], dtv[:, :, :, 1:K + 1],
                    qov[:, :, :, 1:K + 1], Alu.subtract)

                # ---- main scans + output --------------------------------
                for par in (0, 1):
                    ot = opool.tile([HG, BC * W], f32, tag="ot")
                    nc.vector.tensor_tensor_scan(
                        ot[:], atiles[par][0][:],
                        dt[:, par * BC * W:(par + 1) * BC * W],
                        0.0, Alu.mult, Alu.add)
                    nc.sync.dma_start(
                        Od[c0:c0 + BC, par:H:2, :].transpose([1, 0, 2]),
                        ot[:].rearrange("p (c w) -> p c w", c=BC))

    nc.compile()
    return nc


_NC = None


def kernel(X, G1, G2, G3=None, **_):
    global _NC
    if _NC is None:
        _NC = build()
    ue, uo = _upsample_mats()
    in_maps = [
        {"X": np.ascontiguousarray(X[k]), "G1": np.ascontiguousarray(G1[k]),
         "G2": np.ascontiguousarray(G2[k]), "UE": ue, "UO": uo}
        for k in range(NCORES)
    ]
    import os
    res = run_bass_kernel_spmd(_NC, in_maps, list(range(NCORES)),
                               tmpdir=os.environ.get("KERNEL_TMPDIR"))
    kernel.last_result = res
    out = np.stack([res.results[k]["O"] for k in range(NCORES)])
    return out.astype(np.float32, copy=False)


# revision 17
# speedup vs baseline: 1.5635x; 1.0584x over previous
"""GateRecurrent2dnoind (horizontal, forward) Trainium2 kernel, v2.

Semantics (matching the reference):
  G1u, G2u = bilinear 2x upsample (half-pixel) of G1, G2 to (256, 256)
  g1x = G1u * X
  o = g1x; repeat 128x: o = g1x + G2u * shift_right_w(o)   (left edge replicated)

The 128 Jacobi passes equal ONE sequential scan along W:
  s[x] = d[x] + a[x] * s[x-1]      a = G2u, d = G1u*X
with exact depth-128 window emulation (boundary init via geometric series +
a K-column correction scan), identical math to the validated v1 kernel.

v2 engine plan (from measured rates: DVE TT 1.12 ns/el, scan 2.1 ns/el any
dtype/stride, scalar ACTIVATE 1.05 ns/el incl strided+PSUM, gpsimd TT 4.6,
fp32 matmul 592+360(LDW) ns per 512 cols):
  - half-res W-combine m_e[j]=3g[j]+g[j-1], m_o[j]=3g[j]+g[j+1] on DVE (G2)
    and GPSIMD (G1, using a scalar-built 3*g1), from halo'd g tiles.
  - H-upsample = ONE fp32 matmul per (parity, w-parity, 4ch chunk) on PE.
  - scalar engine drains PSUM -> interleaved full-res a~/b~ (strided f32).
  - DVE: d~ = b~*X, correction scan (K cols), then one full-length scan per
    parity writing the output tile directly.

Sharding: batch b -> core b (8 batches, 8 cores). Per core: [64, 256, 256].
"""

import numpy as np

import concourse.bacc as bacc
import concourse.bass as bass
import concourse.mybir as mybir
import concourse.tile as tile
from concourse.bass_utils import run_bass_kernel_spmd

f32 = mybir.dt.float32
f32r = mybir.dt.float32r
Alu = mybir.AluOpType

NCORES = 8
C = 64          # channels per core
H = 256
W = 256
HG = 128        # G input h/w
BC = 8          # channels per block
NBLK = C // BC  # 8
K = 32          # correction columns
SLOT = HG + 2   # g tile slot width (halo col on each side)


def _upsample_mats():
    """lhsT [k=in_row, m=out_row] for the H-upsample matmuls, scaled by 0.25.

    even rows: out[m] = 0.25*in[m-1] + 0.75*in[m]   (m=0 clamps to in[0])
    odd rows:  out[m] = 0.75*in[m] + 0.25*in[m+1]   (m=127 clamps to in[127])
    """
    ue = np.zeros((HG, HG), np.float32)
    uo = np.zeros((HG, HG), np.float32)
    for m in range(HG):
        ue[m, m] += 0.25 * 0.75
        ue[max(m - 1, 0), m] += 0.25 * 0.25
        uo[m, m] += 0.25 * 0.75
        uo[min(m + 1, HG - 1), m] += 0.25 * 0.25
    return ue, uo


def _precompute(nc, pcp, psp, us, G2d):
    """Boundary coefficients per parity from a0 = G2u[..., 0].

    Returns combined [128, 2*C] tiles s0cB, qcB and recbb [128, 2*C*K]:
      s0c = 1 + a0*sum_{m=0}^{127} a0^m          (s[0] = b0*s0c)
      qc  = mask(a0>=0.5) * a0^129               (q-init = b0*qc)
      rec = 1/max(a0, 0.5)  broadcast over K cols into recbb
    """
    g2c0 = pcp.tile([HG, C], f32, tag="g2c0")
    nc.sync.dma_start(g2c0[:], G2d[:, :, 0].transpose([1, 0]))
    s0cB = pcp.tile([HG, 2 * C], f32, tag="s0cB")
    qcB = pcp.tile([HG, 2 * C], f32, tag="qcB")
    recbb = pcp.tile([HG, 2 * C * K], f32, tag="recbb")
    for par in (0, 1):
        ps = psp.tile([HG, C], f32, tag="pc")
        nc.tensor.matmul(ps[:], us[par][:], g2c0[:], start=True, stop=True)
        a0 = pcp.tile([HG, C], f32, tag=f"a0{par}")
        nc.vector.tensor_scalar_mul(a0[:], ps[:], 4.0)
        # geo = sum_{m=0}^{127} a0^m = prod_k (1 + a0^(2^k)), k=0..6
        acc = pcp.tile([HG, C], f32, tag=f"acc{par}")
        p = pcp.tile([HG, C], f32, tag=f"p{par}")
        t = pcp.tile([HG, C], f32, tag=f"t{par}")
        nc.vector.tensor_scalar_add(acc[:], a0[:], 1.0)
        nc.vector.tensor_tensor(p[:], a0[:], a0[:], Alu.mult)
        for _ in range(5):
            nc.vector.tensor_scalar_add(t[:], p[:], 1.0)
            nc.vector.tensor_tensor(acc[:], acc[:], t[:], Alu.mult)
            nc.vector.tensor_tensor(p[:], p[:], p[:], Alu.mult)
        nc.vector.tensor_scalar_add(t[:], p[:], 1.0)
        nc.vector.tensor_tensor(acc[:], acc[:], t[:], Alu.mult)
        a128 = pcp.tile([HG, C], f32, tag=f"a128{par}")
        nc.vector.tensor_tensor(a128[:], p[:], p[:], Alu.mult)
        # s0c = 1 + a0*geo
        nc.vector.tensor_tensor(t[:], a0[:], acc[:], Alu.mult)
        nc.vector.tensor_scalar_add(s0cB[:, par * C:(par + 1) * C], t[:], 1.0)
        # qc = mask(a0>=0.5) * a128 * a0
        mask = pcp.tile([HG, C], f32, tag=f"mask{par}")
        nc.vector.tensor_scalar(mask[:], a0[:], 0.5, None, Alu.is_ge)
        rec = pcp.tile([HG, C], f32, tag=f"rec{par}")
        nc.vector.tensor_scalar_max(t[:], a0[:], 0.5)
        nc.vector.reciprocal(rec[:], t[:])
        qc = pcp.tile([HG, C], f32, tag=f"qc{par}")
        nc.vector.tensor_tensor(qc[:], mask[:], a128[:], Alu.mult)
        nc.vector.tensor_tensor(qcB[:, par * C:(par + 1) * C], qc[:], a0[:],
                                Alu.mult)
        nc.vector.tensor_copy(
            recbb[:].rearrange("p (q c k) -> p q c k", q=2, c=C)[:, par],
            rec[:].unsqueeze(-1).to_broadcast([HG, C, K]))
    return s0cB, qcB, recbb


def build():
    nc = bacc.Bacc("TRN2", target_bir_lowering=False, debug=False,
                   num_devices=NCORES)
    Xd = nc.dram_tensor("X", [C, H, W], f32, kind="ExternalInput")
    G1d = nc.dram_tensor("G1", [C, HG, HG], f32, kind="ExternalInput")
    G2d = nc.dram_tensor("G2", [C, HG, HG], f32, kind="ExternalInput")
    UEd = nc.dram_tensor("UE", [HG, HG], f32, kind="ExternalInput")
    UOd = nc.dram_tensor("UO", [HG, HG], f32, kind="ExternalInput")
    Od = nc.dram_tensor("O", [C, H, W], f32, kind="ExternalOutput")

    with tile.TileContext(nc) as tc:
        with (
            tc.tile_pool(name="const", bufs=1) as constp,
            tc.tile_pool(name="pc", bufs=1) as pcp,
            tc.tile_pool(name="psum", bufs=2, space="PSUM") as psp,
            tc.tile_pool(name="gin", bufs=2) as gpool,
            tc.tile_pool(name="m", bufs=2) as mpool,
            tc.tile_pool(name="ab", bufs=2) as abpool,
            tc.tile_pool(name="x", bufs=2) as xpool,
            tc.tile_pool(name="d", bufs=2) as dpool,
            tc.tile_pool(name="o", bufs=2) as opool,
            tc.tile_pool(name="q", bufs=2) as qpool,
        ):
            ue = constp.tile([HG, HG], f32, tag="ue")
            uo = constp.tile([HG, HG], f32, tag="uo")
            nc.sync.dma_start(ue[:], UEd[:])
            nc.sync.dma_start(uo[:], UOd[:])
            us = (ue, uo)
            uer = constp.tile([HG, HG], f32r, tag="uer")
            uor = constp.tile([HG, HG], f32r, tag="uor")
            nc.vector.tensor_copy(uer[:], ue[:])
            nc.vector.tensor_copy(uor[:], uo[:])
            usr = (uer, uor)

            s0cB, qcB, recbb = _precompute(nc, pcp, psp, us, G2d)
            s0cr = s0cB[:].rearrange("p (q c) -> p q c", q=2)
            qcr = qcB[:].rearrange("p (q c) -> p q c", q=2)
            recr = recbb[:].rearrange("p (q c k) -> p q c k", q=2, c=C)

            for b in range(NBLK):
                c0 = b * BC
                # ---- G loads with halo: slot s=0 -> g[0], 1..128 -> g,
                # s=129 -> g[127] ------------------------------------------
                gts = []
                for Gd, tag in ((G1d, "g1t"), (G2d, "g2t")):
                    gt = gpool.tile([HG, BC * SLOT], f32, tag=tag)
                    gv = gt[:].rearrange("p (c s) -> p c s", c=BC)
                    nc.sync.dma_start(
                        gv[:, :, 1:HG + 1],
                        Gd[c0:c0 + BC, :, :].transpose([1, 0, 2]))
                    nc.sync.dma_start(
                        gv[:, :, 0:1],
                        Gd[c0:c0 + BC, :, 0:1].transpose([1, 0, 2]))
                    nc.sync.dma_start(
                        gv[:, :, SLOT - 1:SLOT],
                        Gd[c0:c0 + BC, :, HG - 1:HG].transpose([1, 0, 2]))
                    gts.append((gt, gv))
                (g1t, g1v), (g2t, g2v) = gts

                # ---- half-res W-combines m_e=3g[j]+g[j-1], m_o=3g[j]+g[j+1]
                # G2 on DVE (STT); G1 on gpsimd (TT with scalar-built 3*g1).
                m2e = mpool.tile([HG, BC * HG], f32r, tag="m2e")
                m2o = mpool.tile([HG, BC * HG], f32r, tag="m2o")
                nc.vector.scalar_tensor_tensor(
                    m2e[:].rearrange("p (c j) -> p c j", c=BC),
                    g2v[:, :, 1:HG + 1], 3.0, g2v[:, :, 0:HG],
                    Alu.mult, Alu.add)
                nc.vector.scalar_tensor_tensor(
                    m2o[:].rearrange("p (c j) -> p c j", c=BC),
                    g2v[:, :, 1:HG + 1], 3.0, g2v[:, :, 2:HG + 2],
                    Alu.mult, Alu.add)
                g1x3 = mpool.tile([HG, BC * SLOT], f32, tag="g1x3")
                nc.scalar.mul(g1x3[:], g1t[:], 3.0)
                g1x3v = g1x3[:].rearrange("p (c s) -> p c s", c=BC)
                m1e = mpool.tile([HG, BC * HG], f32r, tag="m1e")
                m1o = mpool.tile([HG, BC * HG], f32r, tag="m1o")
                nc.gpsimd.tensor_tensor(
                    m1e[:].rearrange("p (c j) -> p c j", c=BC),
                    g1x3v[:, :, 1:HG + 1], g1v[:, :, 0:HG], Alu.add)
                nc.gpsimd.tensor_tensor(
                    m1o[:].rearrange("p (c j) -> p c j", c=BC),
                    g1x3v[:, :, 1:HG + 1], g1v[:, :, 2:HG + 2], Alu.add)

                dt = dpool.tile([HG, 2 * BC * W], f32, tag="dt")
                dtv = dt[:].rearrange("p (q c w) -> p q c w", q=2, c=BC)
                atiles = []
                for par in (0, 1):
                    # ---- PE H-upsample + scalar interleave drain ----------
                    at = abpool.tile([HG, BC * W], f32, tag="at")
                    bt = abpool.tile([HG, BC * W], f32, tag="bt")
                    for mt, dst, eo in ((m2e, at, 0), (m2o, at, 1),
                                        (m1e, bt, 0), (m1o, bt, 1)):
                        dv = dst[:].rearrange(
                            "p (c w2 two) -> p c w2 two", c=BC, two=2)
                        for chunk in range(2):
                            ps = psp.tile([HG, 4 * HG], f32,
                                          tag=f"ps{chunk}")
                            nc.tensor.matmul(
                                ps[:], usr[par][:],
                                mt[:, chunk * 4 * HG:(chunk + 1) * 4 * HG],
                                start=True, stop=True)
                            nc.scalar.copy(
                                dv[:, chunk * 4:(chunk + 1) * 4, :, eo],
                                ps[:].rearrange("p (c j) -> p c j", c=4))
                    # seam restart: a[..., 0] = 0 per channel
                    atv = at[:].rearrange("p (c w) -> p c w", c=BC)
                    nc.gpsimd.memset(atv[:, :, 0:1], 0.0)

                    # ---- X load + d~ = b~ * X ----------------------------
                    xt = xpool.tile([HG, BC * W], f32, tag="xt")
                    nc.sync.dma_start(
                        xt[:].rearrange("p (c w) -> p c w", c=BC),
                        Xd[c0:c0 + BC, par:H:2, :].transpose([1, 0, 2]))
                    nc.vector.tensor_tensor(
                        dt[:, par * BC * W:(par + 1) * BC * W],
                        bt[:], xt[:], Alu.mult)
                    atiles.append((at, atv))

                # ---- corrections (chblock scope, both parities) ----------
                qd = qpool.tile([HG, 2 * BC * (K + 1)], f32, tag="qd")
                qz = qpool.tile([HG, 2 * BC * (K + 1)], f32, tag="qz")
                qo = qpool.tile([HG, 2 * BC * (K + 1)], f32, tag="qo")
                qdv = qd[:].rearrange("p (q c k) -> p q c k", q=2, c=BC)
                qzv = qz[:].rearrange("p (q c k) -> p q c k", q=2, c=BC)
                qov = qo[:].rearrange("p (q c k) -> p q c k", q=2, c=BC)
                nc.gpsimd.memset(qz[:], 0.0)
                nc.gpsimd.memset(qdv[:, :, :, 0:1], 0.0)
                for par in (0, 1):
                    nc.vector.tensor_tensor(
                        qdv[:, par, :, 1:K + 1],
                        atiles[par][1][:, :, 1:K + 1],
                        recr[:, par, c0:c0 + BC, :], Alu.mult)
                # qz spacer col0 = b0*qc (b0 = d~[...,0] pre-overwrite)
                nc.vector.tensor_tensor(
                    qzv[:, :, :, 0:1], dtv[:, :, :, 0:1],
                    qcr[:, :, c0:c0 + BC].unsqueeze(-1), Alu.mult)
                # d~ col0 = b0 * s0c (after qz spacer read)
                nc.vector.tensor_tensor(
                    dtv[:, :, :, 0:1], dtv[:, :, :, 0:1],
                    s0cr[:, :, c0:c0 + BC].unsqueeze(-1), Alu.mult)
                nc.vector.tensor_tensor_scan(
                    qo[:], qd[:], qz[:], 0.0, Alu.mult, Alu.add)
                nc.vector.tensor_tensor(
                    dtv[:, :, :, 1:K + 1], dtv[:, :, :, 1:K + 1],
                    qov[:, :, :, 1:K + 1], Alu.subtract)

                # ---- main scans + output --------------------------------
                for par in (0, 1):
                    ot = opool.tile([HG, BC * W], f32, tag="ot")
                    nc.vector.tensor_tensor_scan(
                        ot[:], atiles[par][0][:],
                        dt[:, par * BC * W:(par + 1) * BC * W],
                        0.0, Alu.mult, Alu.add)
                    nc.sync.dma_start(
                        Od[c0:c0 + BC, par:H:2, :].transpose([1, 0, 2]),
                        ot[:].rearrange("p (c w) -> p c w", c=BC))

    nc.compile()
    return nc


_NC = None


def kernel(X, G1, G2, G3=None, **_):
    global _NC
    if _NC is None:
        _NC = build()
    ue, uo = _upsample_mats()
    in_maps = [
        {"X": np.ascontiguousarray(X[k]), "G1": np.ascontiguousarray(G1[k]),
         "G2": np.ascontiguousarray(G2[k]), "UE": ue, "UO": uo}
        for k in range(NCORES)
    ]
    import os
    res = run_bass_kernel_spmd(_NC, in_maps, list(range(NCORES)),
                               tmpdir=os.environ.get("KERNEL_TMPDIR"))
    kernel.last_result = res
    out = np.stack([res.results[k]["O"] for k in range(NCORES)])
    return out.astype(np.float32, copy=False)
